# revision 79
# baseline (speedup 1.0000x reference)
"""AttentionAugmentation2D kernel for 8 Trainium2 NeuronCores.

Data-parallel over batch (B=8 -> 1 batch element per core).

Math (per batch, per head; H=W=32, L=H*W=1024, dh=32):
  logits[(x,y),(x',y')] = q.k + q.krw[y'-y+31] + q.krh[x'-x+31]
Both relative terms are folded into a single K=96 matmul:
  Q_aug = [qT; skew_w(q @ krw^T); skew_h(q @ krh^T)]   (96 x 1024 per head)
  K_aug = [kT; onehot32(y'); onehot32(x')]             (96 x 1024 per head)
logits are computed transposed (keys on partitions) so that exp(logitsT)
is directly the rhs of the attention@V matmul. Softmax denominator comes
from a ones-column appended to V; division happens after transposing the
per-head [33,1024] result back to [q, d] layout.
"""

import numpy as np

import concourse.bass as bass
import concourse.mybir as mybir
import concourse.tile as tile
from concourse import bacc
from concourse.bass_utils import run_bass_kernel_spmd

FP = mybir.dt.float32
FPR = mybir.dt.float32r
AF = mybir.ActivationFunctionType


B = 8
H = W = 32
NH = 8
DH = 32          # per-head depth for q/k/v
L = H * W        # 1024 positions
SCALE = float(DH) ** -0.5
NT = L // 128    # 8 position tiles


def _build_onehot():
    # rows 0-31: onehot of y' = key % 32 ; rows 32-63: onehot of x' = key // 32
    # bf16: 0/1 are exact, halves the transfer; SWDGE casts to f32r on write
    import ml_dtypes

    oh = np.zeros((64, L), dtype=np.float32)
    k = np.arange(L)
    oh[k % 32, k] = 1.0
    oh[32 + k // 32, k] = 1.0
    return np.ascontiguousarray(np.tile(oh, (1, NH)).astype(ml_dtypes.bfloat16))


def _build_nc():
    nc = bacc.Bacc(
        "TRN2",
        target_bir_lowering=False,
        debug=False,
        enable_asserts=True,
        num_devices=B,
    )
    x = nc.declare_dram_parameter("x", [L, 3 * NH * DH], FP, isOutput=False)
    krw = nc.declare_dram_parameter("krwT", [DH, 2 * W - 1], FPR, isOutput=False)
    krh = nc.declare_dram_parameter("krhT", [DH, 2 * H - 1], FPR, isOutput=False)
    oneh = nc.declare_dram_parameter("oneh", [64, NH * L], mybir.dt.bfloat16, isOutput=False)
    eye = nc.declare_dram_parameter("eye", [128, 128], FP, isOutput=False)
    out = nc.declare_dram_parameter("out", [L, NH * DH], FP, isOutput=True)

    with tile.TileContext(nc) as tc:
        with (
            tc.tile_pool(name="const", bufs=1) as cp,
        ):
            ident = cp.tile([128, 128], FP)
            nc.scalar.dma_start(out=ident, in_=eye[:])
            krw_sb = cp.tile([DH, 2 * W - 1], FPR)
            krh_sb = cp.tile([DH, 2 * H - 1], FPR)

            QaugT = cp.tile([96, NH * L], FPR)
            KaugT = cp.tile([96, NH * L], FPR)
            # constant onehot rows (same for every head; pre-tiled on host).
            # SWDGE queue: keeps the big x load unblocked on the HWDGE ring.
            nc.gpsimd.dma_start(out=KaugT[32:96, :], in_=oneh[:])

            Vaug = cp.tile([128, NT, NH, DH + 1], FPR)
            # ones column for the softmax denominator: broadcast-DMA of
            # eye[0,0] (f32r memset has no ISA encoding; SWDGE casts)
            ones_src = bass.AP(
                tensor=eye, offset=0, ap=[[0, 128], [0, NT * NH]]
            )
            ones_dst = Vaug[:, :, :, DH : DH + 1].rearrange("p t h o -> p (t h o)")
            nc.gpsimd.dma_start(out=ones_dst, in_=ones_src)

            out_sb = cp.tile([128, NT, NH * DH], FP)

            # ---------------- setup: transposes, rel logits, skew ----------
            with (
                tc.tile_pool(name="setup", bufs=1) as sp,
                tc.tile_pool(name="ps_tr", bufs=2, space="PSUM") as ps_tr,
                tc.tile_pool(name="ps_rel", bufs=3, space="PSUM") as ps_rel,
            ):
                # per-chunk input tiles so each transpose depends only on its
                # own 512KB DMA, not the whole 3MB load
                xr = x.rearrange("(t p) c -> p t c", p=128)
                chunks = [
                    sp.tile([128, NT, 128], FP, tag=f"in{j}", name=f"chunk{j}")
                    for j in range(6)
                ]
                # FIFO ring order: q/k chunks first (transposes), then the
                # small rel tables, then the v chunks (needed latest)
                # first q-chunk lands in two halves so the first four
                # transposes can start ~0.8us earlier
                nc.sync.dma_start(
                    out=chunks[0][:, 0:4, :], in_=xr[:, 0:4, 0:128]
                )
                nc.sync.dma_start(
                    out=chunks[0][:, 4:8, :], in_=xr[:, 4:8, 0:128]
                )
                nc.sync.dma_start(
                    out=chunks[1][:, 0:4, :], in_=xr[:, 0:4, 128:256]
                )
                nc.sync.dma_start(
                    out=chunks[1][:, 4:8, :], in_=xr[:, 4:8, 128:256]
                )
                for j in (2, 3, 4, 5):
                    nc.sync.dma_start(
                        out=chunks[j], in_=xr[:, :, j * 128 : (j + 1) * 128]
                    )
                nc.scalar.dma_start(out=krw_sb, in_=krw[:])
                nc.scalar.dma_start(out=krh_sb, in_=krh[:])
                qT = sp.tile([128, 2 * L], FPR)   # channels 0-255 transposed
                kT = sp.tile([128, 2 * L], FPR)   # channels 256-511 transposed
                for j in range(4):
                    dst = qT if j < 2 else kT
                    jj = j % 2
                    for t in range(NT):
                        tp = ps_tr.tile([128, 128], FP, tag="tp")
                        nc.tensor.transpose(tp, chunks[j][:, t, :], ident)
                        seg = dst[:, jj * L + t * 128 : jj * L + (t + 1) * 128]
                        if j < 2:
                            # fold in the 1/sqrt(dh) scaling of q; alternate
                            # engines to halve the serial evacuation chain
                            if t % 2 == 0:
                                nc.scalar.mul(seg, tp, SCALE)
                            else:
                                nc.vector.tensor_scalar_mul(seg, tp, SCALE)
                        else:
                            if t % 2 == 0:
                                nc.vector.tensor_copy(seg, tp)
                            else:
                                nc.scalar.copy(seg, tp)

                # build V (+ ones column) from the already-loaded input;
                # gpsimd is free once the onehot DMA drains, and V is not
                # needed until the first attention@V (~22us)
                for j in (4, 5):
                    nc.gpsimd.tensor_copy(
                        Vaug[:, :, 4 * (j - 4) : 4 * (j - 3), 0:DH],
                        chunks[j].rearrange("p t (h d) -> p t h d", d=DH),
                    )

                # scatter per-head rows into the augmented tensors (row 0-31).
                # q first (the rel matmuls wait on all of them); k rows are
                # only needed by the first QK matmul, later. Spread across
                # engines to shorten the serial chain.
                def row_copy(dst_aug, src, h):
                    r0 = (h % 4) * 32
                    c0 = (h // 4) * L
                    eng = nc.vector if h < 5 else (nc.vector, nc.scalar)[h % 2]
                    if eng is nc.scalar:
                        eng.copy(
                            dst_aug[0:32, h * L : (h + 1) * L],
                            src[r0 : r0 + 32, c0 : c0 + L],
                        )
                    else:
                        eng.tensor_copy(
                            dst_aug[0:32, h * L : (h + 1) * L],
                            src[r0 : r0 + 32, c0 : c0 + L],
                        )

                for h in range(NH):
                    row_copy(QaugT, qT, h)
                # k rows ride the otherwise-idle gpsimd engine; only the
                # first QK matmul (well after the rel phase) needs them
                for h in range(NH):
                    r0 = (h % 4) * 32
                    c0 = (h // 4) * L
                    nc.gpsimd.tensor_copy(
                        KaugT[0:32, h * L : (h + 1) * L],
                        kT[r0 : r0 + 32, c0 : c0 + L],
                    )

                # relative logits, pre-skewed: for queries with fixed y,
                #   QaugT[32+y', (h,x,y)] = sum_d q[d,(h,x,y)] * krw[d, y'-y+31]
                # i.e. lhsT = krwT free-slice [:, 31-y : 63-y] (free offsets are
                # unrestricted, unlike partition offsets). One matmul per y
                # covers all heads (N = NH*H = 256).
                qrows_w = QaugT[0:32, :].rearrange(
                    "p (h x y2) -> p h x y2", x=H, y2=W
                )
                qw_dst = QaugT[32:64, :].rearrange(
                    "p (h x y2) -> p h x y2", x=H, y2=W
                )
                qh_dst = QaugT[64:96, :].rearrange(
                    "p (h x2 y) -> p h x2 y", x2=H, y=W
                )
                qrows_h = QaugT[0:32, :].rearrange(
                    "p (h x2 y) -> p h x2 y", x2=H, y=W
                )
                for g in range(W // 4):
                    rp = ps_rel.tile([32, 4, NH * H], FP, tag="rp")
                    for i in range(4):
                        y = 4 * g + i
                        nc.tensor.matmul(
                            rp[:, i, :],
                            lhsT=krw_sb[:, 31 - y : 63 - y],
                            rhs=qrows_w[:, :, :, y],
                            start=True,
                            stop=True,
                        )
                    ev = rp.rearrange("p i (h x) -> p i h x", h=NH)
                    dst = qw_dst[:, :, :, 4 * g : 4 * g + 4].rearrange(
                        "p h x i -> p i h x"
                    )
                    if g % 2 == 0:
                        nc.vector.tensor_copy(dst, ev)
                    else:
                        nc.scalar.copy(dst, ev)
                # QaugT[64+x', (h,x,y)] = sum_d q[d,(h,x,y)] * krh[d, x'-x+31]
                for g in range(H // 4):
                    rp = ps_rel.tile([32, 4, NH * W], FP, tag="rp")
                    for i in range(4):
                        xx = 4 * g + i
                        nc.tensor.matmul(
                            rp[:, i, :],
                            lhsT=krh_sb[:, 31 - xx : 63 - xx],
                            rhs=qrows_h[:, :, xx, :],
                            start=True,
                            stop=True,
                        )
                    ev = rp.rearrange("p i (h y) -> p i h y", h=NH)
                    dst = qh_dst[:, :, 4 * g : 4 * g + 4, :].rearrange(
                        "p h i y -> p i h y"
                    )
                    if g % 2 == 0:
                        nc.vector.tensor_copy(dst, ev)
                    else:
                        nc.scalar.copy(dst, ev)

            # ---------------- attention over heads ------------------------
            with (
                tc.tile_pool(name="wt", bufs=2) as wtp,
                tc.tile_pool(name="at", bufs=2) as atp,
                tc.tile_pool(name="sm", bufs=8) as smp,
                tc.tile_pool(name="ps_lt", bufs=2, space="PSUM") as ps_lt,
                tc.tile_pool(name="ps_av", bufs=2, space="PSUM") as ps_av,
                tc.tile_pool(name="ps_ft", bufs=2, space="PSUM") as ps_ft,
            ):
                wts = {}
                avs = {}

                def finish_head(h):
                    WT = wts.pop(h)
                    del WT
                    av0, av1 = avs.pop(h)
                    at_sb = atp.tile([DH + 1, L], FP, tag="at")
                    nc.vector.tensor_copy(at_sb[:, 0:512], av0)
                    if h == NH - 1:
                        # ACT is idle after the final exp: parallel evac
                        nc.scalar.copy(at_sb[:, 512:1024], av1)
                    else:
                        nc.vector.tensor_copy(at_sb[:, 512:1024], av1)

                    last = h == NH - 1
                    for t in range(NT):
                        ft = ps_ft.tile([128, DH + 1], FP, tag="ft")
                        nc.tensor.transpose(
                            ft,
                            at_sb[:, t * 128 : (t + 1) * 128],
                            ident[0 : DH + 1, 0 : DH + 1],
                        )
                        rcp = smp.tile([128, 1], FP, tag="rcp")
                        nc.vector.reciprocal(rcp, ft[:, DH : DH + 1])
                        if last and t % 2 == 1:
                            # ACT is idle after the final exp: share the tail
                            nc.scalar.mul(
                                out_sb[:, t, h * DH : (h + 1) * DH],
                                ft[:, 0:DH],
                                rcp,
                            )
                        else:
                            nc.vector.tensor_scalar_mul(
                                out_sb[:, t, h * DH : (h + 1) * DH], ft[:, 0:DH], rcp
                            )

                # software pipeline: head h's QK+exp interleaves kt-wise with
                # head h-1's attention@V, so the PE never sits idle between
                # exp bursts and the last head's AV tail is one kt deep.
                for h in range(NH + 1):
                    if h < NH:
                        c0 = h * L
                        wts[h] = wtp.tile([128, NT * L], FPR, tag="wt", name=f"wt{h}")
                    if h > 0:
                        avs[h - 1] = (
                            ps_av.tile([DH + 1, 512], FP, tag="av", name=f"av{h}a"),
                            ps_av.tile([DH + 1, 512], FP, tag="av", name=f"av{h}b"),
                        )
                    for kt in range(NT):
                        if h < NH:
                            lt = ps_lt.tile([128, L], FP, tag="lt")
                            for qc in range(2):
                                nc.tensor.matmul(
                                    lt[:, qc * 512 : (qc + 1) * 512],
                                    lhsT=KaugT[:, c0 + kt * 128 : c0 + (kt + 1) * 128],
                                    rhs=QaugT[:, c0 + qc * 512 : c0 + (qc + 1) * 512],
                                    start=True,
                                    stop=True,
                                )
                            nc.scalar.activation(
                                wts[h][:, kt * L : (kt + 1) * L], lt[:, :], AF.Exp
                            )
                        if h > 0:
                            WTp = wts[h - 1]
                            for qc in range(2):
                                nc.tensor.matmul(
                                    avs[h - 1][qc],
                                    lhsT=Vaug[:, kt, h - 1, :],
                                    rhs=WTp[:, kt * L + qc * 512 : kt * L + (qc + 1) * 512],
                                    start=(kt == 0),
                                    stop=(kt == NT - 1),
                                )
                    if h > 0:
                        finish_head(h - 1)

            out_r = out.rearrange("(t p) c -> p t c", p=128)
            for t in range(NT):
                # alternate the two HWDGE rings so the 8 stores drain in half
                # the serial time at the kernel tail
                eng = nc.sync if t % 2 == 0 else nc.scalar
                eng.dma_start(out=out_r[:, t, :], in_=out_sb[:, t, :])
    nc.compile()
    return nc


_NC_CACHE = None


def kernel(inputs: np.ndarray, key_rel_w: np.ndarray, key_rel_h: np.ndarray) -> np.ndarray:
    global _NC_CACHE
    x = np.ascontiguousarray(inputs.astype(np.float32).reshape(B, L, 3 * NH * DH))
    krwT = np.ascontiguousarray(key_rel_w.astype(np.float32).T)
    krhT = np.ascontiguousarray(key_rel_h.astype(np.float32).T)
    oneh = _build_onehot()

    if _NC_CACHE is None:
        _NC_CACHE = _build_nc()
    nc = _NC_CACHE

    eye = np.eye(128, dtype=np.float32)
    in_maps = [
        {"x": x[b], "krwT": krwT, "krhT": krhT, "oneh": oneh, "eye": eye}
        for b in range(B)
    ]
    res = run_bass_kernel_spmd(nc, in_maps, list(range(B)))
    out = np.stack([res.results[b]["out"] for b in range(B)], axis=0)
    return np.ascontiguousarray(out.reshape(B, H, W, NH * DH).astype(np.float32))
